# revision 18
# baseline (speedup 1.0000x reference)
"""HRM dense-transformer kernel for 8 trn2 NeuronCores.

Sharding: data-parallel over batch (4) x sequence-parallel (2).
Core c handles batch b=c//2, token half h=c%2 (512 tokens).
Per block each core computes q/k/v for its own tokens, all-gathers
k^T and v (within its pair), then computes attention for its 512
queries over all 1024 keys. Activations are feature-major
([feature(part), token(free)]); scores are computed transposed [tk, tq]
so the softmax sum is a ones-matmul partition reduction. V is computed
token-major by swapping matmul operands, with a ones column appended
(M=65) so the softmax denominator falls out of the PV matmul.

Perf structure (v3):
- residual stream is fp16 (zL/zH/emb/hb); fp32 only in the pre-norm
  scratch hs. Norm mults write the f16 state directly (no casts), split
  across DVE and Pool.
- rope: ACT copies the projection psum to fp16 SBUF, then all-fp16
  elementwise ops run at 2x split across DVE+Pool.
- scores land in [128,2,512] 2-bank PSUM chunks, exp'd in [128,1024]
  batches. Softmax normalize is deferred per head-pair: denominators at
  partitions 0/32, fast-recip, one bf16 K=64 selector matmul broadcast
  (bf16 for range: 1/den underflows fp16).
- rmsnorm: square on Pool (keeps ACT tables stable), Sqrt on ACT +
  fast-recip, fp16 K=1 broadcast matmul.
- ACT table switches (Exp/Sqrt/Silu) are prefetched with dummy [1,1]
  activations during PE-heavy phases.
- weights pre-tiled host-side: every weight DMA reads 2KB contiguous
  per partition; gate+up column blocks interleave into one DMA.
"""

import os
import sys

sys.path.insert(0, "/opt/trn_rl_repo")

import ml_dtypes
import numpy as np

import concourse.bass as bass
import concourse.mybir as mybir
import concourse.tile as tile
from concourse import bacc
from concourse.bass_utils import run_bass_kernel_spmd

F32 = mybir.dt.float32
F16 = mybir.dt.float16
BF16 = mybir.dt.bfloat16
AF = mybir.ActivationFunctionType
MUL = mybir.AluOpType.mult
ADD = mybir.AluOpType.add

B, S, D, NH, HD = 4, 1024, 1024, 16, 64
INTER = 2816
T = S // 2              # own tokens per core
DT = D // 128           # 8 d-tiles
IT = INTER // 128       # 22 inter tiles
VF = NH * (HD + 1)      # 1040, v_aug feature width
EPS = 1e-5
KN = T * S              # kT elems (own): 1024 x 512
VN = T * VF             # v_aug elems (own): 512 x 1040
RG = [[0, 1], [2, 3], [4, 5], [6, 7]]

N_LEVEL_CALLS = int(os.environ.get("HRM_LEVEL_CALLS", "6"))

_CACHE = {}


def build_kernel():
    nc = bacc.Bacc("TRN2", target_bir_lowering=False, debug=False, num_devices=8)

    inp = {}
    for nm, shape, dt in [
        ("zL", [D, T], F16), ("zH", [D, T], F16), ("emb", [D, T], F16),
        ("cosT", [128, T], F16), ("sinxT", [128, T], F16),
        # pre-tiled weights: [layer, out-tile, 128(p=in), in-tile, m]
        ("L_wqT", [2, DT, 128, DT, 128], F16),
        ("L_wkT", [2, DT, 128, DT, 128], F16),
        ("L_wvT", [2, 4, 128, DT, 256], F16),
        ("L_woT", [2, DT, 128, DT, 128], F16),
        ("L_guT", [2, IT, 128, DT, 256], F16),
        ("L_dnT", [2, DT, 128, IT, 128], F16),
        ("H_wqT", [2, DT, 128, DT, 128], F16),
        ("H_wkT", [2, DT, 128, DT, 128], F16),
        ("H_wvT", [2, 4, 128, DT, 256], F16),
        ("H_woT", [2, DT, 128, DT, 128], F16),
        ("H_guT", [2, IT, 128, DT, 256], F16),
        ("H_dnT", [2, DT, 128, IT, 128], F16),
    ]:
        inp[nm] = nc.dram_tensor(nm, shape, dt, kind="ExternalInput")
    out_t = nc.dram_tensor("zH_out", [D, T], F16, kind="ExternalOutput")

    seq = os.environ.get("HRM_SEQ", "")
    if seq:
        level_calls = list(seq)
    else:
        level_calls = (["L", "L", "H"] * 2)[:N_LEVEL_CALLS]

    with tile.TileContext(nc) as tc:
        with (
            tc.tile_pool(name="state", bufs=1) as st,
            tc.tile_pool(name="sp", bufs=2) as sp,
            tc.tile_pool(name="big", bufs=2) as bigp,
            tc.tile_pool(name="w128", bufs=3) as w128p,
            tc.tile_pool(name="w256", bufs=2) as w256p,
            tc.tile_pool(name="wd", bufs=2) as wdp,
            tc.tile_pool(name="pt", bufs=2) as ptp,
            tc.tile_pool(name="psum", bufs=1, space="PSUM") as psum,
            tc.tile_pool(name="dram", bufs=2, space="DRAM") as dram,
        ):
            zL = st.tile([128, DT, T], F16, name="zL_sb")
            zH = st.tile([128, DT, T], F16, name="zH_sb")
            emb = st.tile([128, DT, T], F16, name="emb_sb")
            hs = st.tile([128, DT, T], F32, name="hs")
            cos = st.tile([128, T], F16, name="cos_sb")
            sinx = st.tile([128, T], F16, name="sinx_sb")
            hb = st.tile([128, DT, T], F16, name="hb")
            qT = st.tile([128, DT, T], F16, name="qT")
            kst = st.tile([128, DT, T], F16, name="kst")
            vst = st.tile([128, 4, VF], BF16, name="vst")
            oT = st.tile([128, DT, T], F16, name="oT")
            ones128 = st.tile([128, 1], F16, name="ones128")
            ones1f = st.tile([1, 128], F16, name="ones1f")
            sel64 = st.tile([64, 128], BF16, name="sel64")
            den2 = st.tile([64, 512], F32, name="den2")
            denr = st.tile([64, 512], F32, name="denr")
            denr16 = st.tile([64, 512], BF16, name="denr16")
            epsc = st.tile([1, 1], F32, name="epsc")
            dum = st.tile([1, 1], F32, name="dum")

            nc.sync.dma_start(zL[:], inp["zL"].rearrange("(dt p) t -> p dt t", p=128))
            nc.sync.dma_start(zH[:], inp["zH"].rearrange("(dt p) t -> p dt t", p=128))
            nc.sync.dma_start(emb[:], inp["emb"].rearrange("(dt p) t -> p dt t", p=128))
            nc.sync.dma_start(cos[:], inp["cosT"][:])
            nc.sync.dma_start(sinx[:], inp["sinxT"][:])
            nc.vector.memset(ones128[:], 1.0)
            nc.vector.memset(ones1f[:], 1.0)
            nc.vector.memset(epsc[:], EPS)
            nc.vector.memset(dum[:], 1.0)
            # selector for the softmax-denominator broadcast: row 0 feeds
            # out partitions 0-63 (even head), row 32 feeds 64-127 (odd).
            # den rows sit at partitions 0/32 (engine writes need 32-aligned
            # partition bases); unused rows stay at the 1.0 init so the
            # zero-weighted matmul columns never see NaN/Inf.
            nc.vector.memset(sel64[:], 0.0)
            nc.vector.memset(sel64[0:1, 0:64], 1.0)
            nc.vector.memset(sel64[32:33, 64:128], 1.0)
            nc.vector.memset(den2[:], 1.0)
            nc.vector.memset(denr[:], 1.0)
            nc.vector.memset(denr16[:], 1.0)
            # ones columns of v_aug (written once; data copies avoid them)
            nc.vector.memset(
                vst.rearrange("p tt (h c) -> p tt h c", c=HD + 1)[:, :, :, HD : HD + 1],
                1.0,
            )

            def preload(func):
                # dummy activation to pull the act table in early
                nc.scalar.activation(dum[:], dum[:], func)

            def _rope(ps, out_ap, pool_assist=False):
                """out = ps*cos + rotate_half(ps)*sin.

                partition dim = 2 heads x 64; sinT rows pre-negated for the
                first half of each head. ACT stages the psum to fp16 so the
                elementwise ops run all-fp16 at 2x, split DVE/Pool.
                """
                qr = sp.tile([128, 512], F16, tag="qr", name="qr")
                nc.scalar.copy(qr[:], ps[:])
                t1 = sp.tile([128, 512], F16, tag="rope1", name="rope1")
                t2 = sp.tile([128, 512], F16, tag="rope2", name="rope2")
                nc.vector.tensor_tensor(t1[:], qr[:], cos[:], MUL)
                nc.vector.tensor_tensor(t2[0:32, :], qr[32:64, :],
                                        sinx[32:64, :], MUL)
                nc.vector.tensor_tensor(t2[32:64, :], qr[0:32, :],
                                        sinx[0:32, :], MUL)
                e2 = nc.gpsimd if pool_assist else nc.vector
                e2.tensor_tensor(t2[64:96, :], qr[96:128, :],
                                 sinx[96:128, :], MUL)
                e2.tensor_tensor(t2[96:128, :], qr[64:96, :],
                                 sinx[64:96, :], MUL)
                if pool_assist:
                    nc.gpsimd.tensor_tensor(out_ap, t1[:], t2[:], ADD)
                else:
                    nc.vector.tensor_add(out=out_ap, in0=t1[:], in1=t2[:])

            def _rmsnorm(out_tile):
                """out = hs * rsqrt(mean(hs^2)+eps) in fp16.

                square on Pool (keeps ACT table stable), partition-reduce via
                ones matmul, Sqrt on ACT + fast reciprocal, fp16 K=1
                broadcast matmul, mults split DVE/Pool."""
                sst = psum.tile([65, 512], F32, tag="pv", bufs=2, name="sst")
                ss = sst[0:1, :]
                for dt in range(DT):
                    r2 = sp.tile([128, 512], F16, tag="r2", name="r2")
                    nc.gpsimd.tensor_tensor(r2[:], hs[:, dt, :], hs[:, dt, :],
                                            MUL)
                    nc.tensor.matmul(ss, ones128[:], r2[:], start=(dt == 0),
                                     stop=(dt == DT - 1))
                sq = sp.tile([1, 512], F32, tag="sq", bufs=1, name="sq")
                nc.scalar.activation(sq[:], ss, AF.Sqrt, bias=epsc[:],
                                     scale=1.0 / D)
                rstdf = sp.tile([1, 512], F32, tag="rstdf", bufs=1, name="rstdf")
                nc.vector.reciprocal_approx_fast(out=rstdf[:], in_=sq[:])
                rstd = sp.tile([1, 512], F16, tag="rstd", bufs=1, name="rstd")
                nc.vector.tensor_copy(out=rstd[:], in_=rstdf[:])
                pb = psum.tile([128, 3, 512], F32, tag="sc", bufs=2, name="pbn")[:, 0, :]
                nc.tensor.matmul(pb[:], ones1f[:], rstd[:], start=True, stop=True)
                bc = sp.tile([128, 512], F16, tag="bcn", bufs=1, name="bcn")
                nc.vector.tensor_copy(out=bc[:], in_=pb[:])
                for dt in range(DT):
                    eng = nc.vector if dt % 2 == 0 else nc.gpsimd
                    eng.tensor_tensor(out_tile[:, dt, :], hs[:, dt, :], bc[:],
                                      MUL)

            def block(wq, wk, wv, wo, gu, dn, mid_out, end_out):
                """One HRMBlock on hb; attn rmsnorm -> mid_out(=hb),
                mlp rmsnorm -> end_out (hb or the level state tile)."""
                gin_k = dram.tile([KN], F16, tag="gin_k", name="gin_k")
                gout_k = dram.tile([2 * KN], F16, tag="gout_k", name="gout_k")
                gin_v = dram.tile([VN], BF16, tag="gin_v", name="gin_v")
                gout_v = dram.tile([2 * VN], BF16, tag="gout_v", name="gout_v")

                # ---- k projection + rope ----
                for ot in range(DT):
                    w = w128p.tile([128, DT, 128], F16, tag="w128", name="wk")
                    nc.sync.dma_start(w[:], wk[ot])
                    ps = psum.tile([128, 3, 512], F32, tag="sc", bufs=2, name="psk")[:, 0, :]
                    for dt in range(DT):
                        nc.tensor.matmul(ps[:], w[:, dt, :], hb[:, dt, :],
                                         start=(dt == 0), stop=(dt == DT - 1))
                    _rope(ps, kst[:, ot, :], pool_assist=True)
                # ---- send + gather k (overlaps v/q projection) ----
                nc.sync.dma_start(
                    gin_k[:].rearrange("(dt p t) -> p dt t", p=128, t=T), kst[:])
                nc.gpsimd.collective_compute(
                    "AllGather", mybir.AluOpType.bypass, replica_groups=RG,
                    ins=[gin_k.opt()], outs=[gout_k.opt()])
                # ---- v projection (token-major) ----
                vsr = vst.rearrange("p tt (hh c) -> p tt hh c", c=HD + 1)
                for oc in range(4):
                    w = w256p.tile([128, DT, 256], F16, tag="w256", name="wv")
                    nc.sync.dma_start(w[:], wv[oc])
                    for tt in range(4):
                        ps = psum.tile([128, 3, 512], F32, tag="sc", bufs=2,
                                       name="psv")[:, 0, 0:256]
                        for dt in range(DT):
                            nc.tensor.matmul(
                                ps[:], hb[:, dt, tt * 128 : (tt + 1) * 128],
                                w[:, dt, :], start=(dt == 0), stop=(dt == DT - 1))
                        nc.vector.tensor_copy(
                            out=vsr[:, tt, oc * 4 : (oc + 1) * 4, 0:HD],
                            in_=ps.rearrange("p (hh c) -> p hh c", c=HD))
                # ---- send + gather v (k already in flight) ----
                nc.sync.dma_start(
                    gin_v[:].rearrange("(tt p f) -> p tt f", p=128, f=VF), vst[:])
                nc.gpsimd.collective_compute(
                    "AllGather", mybir.AluOpType.bypass, replica_groups=RG,
                    ins=[gin_v.opt()], outs=[gout_v.opt()])
                # ---- q projection + rope (overlaps gather) ----
                for ot in range(DT):
                    w = w128p.tile([128, DT, 128], F16, tag="w128", name="wq")
                    nc.sync.dma_start(w[:], wq[ot])
                    ps = psum.tile([128, 3, 512], F32, tag="sc", bufs=2, name="psq")[:, 0, :]
                    for dt in range(DT):
                        nc.tensor.matmul(ps[:], w[:, dt, :], hb[:, dt, :],
                                         start=(dt == 0), stop=(dt == DT - 1))
                    _rope(ps, qT[:, ot, :])
                # ---- load gathered k/v ----
                kTf = bigp.tile([128, DT, S], F16, tag="big", name="kTf")
                vf = bigp.tile([128, DT, VF], BF16, tag="big", name="vf")
                for r in range(2):
                    nc.sync.dma_start(
                        kTf[:, :, r * T : (r + 1) * T],
                        gout_k[r * KN : (r + 1) * KN].rearrange(
                            "(dt p t) -> p dt t", p=128, t=T))
                    nc.sync.dma_start(
                        vf[:, 4 * r : 4 * r + 4, :],
                        gout_v[r * VN : (r + 1) * VN].rearrange(
                            "(tt p f) -> p tt f", p=128, f=VF))
                # ---- attention ----
                for ot in range(DT):
                    # flat [slot=(kt,sub)] prob tile; score pairs (sub0,sub1)
                    # at one kt run concurrently on PE row-quadrants
                    # (0,0)+(64,0); exp batches 3 slots (3 psum banks) to
                    # amortize ACT per-op overhead.
                    pt = ptp.tile([128, 16, 512], BF16, tag="pt", bufs=2,
                                  name="pt")
                    sc = None
                    c0 = 0
                    for s in range(16):
                        kt, sub = s // 2, s % 2
                        lane = s - c0
                        if sc is None:
                            sc = psum.tile([128, 3, 512], F32, tag="sc",
                                           bufs=2, name="sc")
                        bp = sub * 64
                        nc.tensor.matmul(
                            sc[:, lane, :],
                            kTf[bp : bp + 64, ot, kt * 128 : (kt + 1) * 128],
                            qT[bp : bp + 64, ot, :],
                            start=True, stop=True, tile_position=(bp, 0))
                        if lane == 2 or s == 15:
                            nc.scalar.activation(
                                pt[:, c0 : s + 1, :], sc[:, 0 : lane + 1, :],
                                AF.Exp, scale=0.125)
                            sc = None
                            c0 = s + 1
                    pvs = []
                    for sub in range(2):
                        hh = ot * 2 + sub
                        pv = psum.tile([65, 512], F32, tag="pv", bufs=2, name="pv")
                        for kt in range(DT):
                            nc.tensor.matmul(
                                pv[:],
                                vf[:, kt, hh * (HD + 1) : (hh + 1) * (HD + 1)],
                                pt[:, kt * 2 + sub, :],
                                start=(kt == 0), stop=(kt == DT - 1))
                        nc.vector.tensor_copy(out=den2[sub * 32 : sub * 32 + 1, :],
                                              in_=pv[64:65, :])
                        pvs.append(pv)
                    if ot == DT - 1:
                        preload(AF.Sqrt)
                    nc.vector.reciprocal_approx_fast(out=denr[:], in_=den2[:])
                    nc.vector.tensor_copy(out=denr16[:], in_=denr[:])
                    pb = psum.tile([128, 3, 512], F32, tag="sc", bufs=2, name="pbc")[:, 0, :]
                    nc.tensor.matmul(pb[:], sel64[:], denr16[:], start=True,
                                     stop=True)
                    bc = sp.tile([128, 512], BF16, tag="bc", bufs=1, name="bc")
                    nc.vector.tensor_copy(out=bc[:], in_=pb[:])
                    for sub in range(2):
                        bp = sub * 64
                        nc.vector.tensor_tensor(
                            oT[bp : bp + 64, ot, :], pvs[sub][0:HD, :],
                            bc[bp : bp + 64, :], MUL)
                # ---- o projection + residual ----
                for dt2 in range(DT):
                    w = w128p.tile([128, DT, 128], F16, tag="w128", name="wo")
                    nc.sync.dma_start(w[:], wo[dt2])
                    ps = psum.tile([128, 3, 512], F32, tag="sc", bufs=2, name="pso")[:, 0, :]
                    for et in range(DT):
                        nc.tensor.matmul(ps[:], w[:, et, :], oT[:, et, :],
                                         start=(et == 0), stop=(et == DT - 1))
                    nc.vector.tensor_tensor(hs[:, dt2, :], hb[:, dt2, :],
                                            ps[:], ADD)
                _rmsnorm(mid_out)
                preload(AF.Silu)
                # ---- MLP ----
                act = bigp.tile([128, IT, 512], F16, tag="big", name="act")
                for it in range(IT):
                    wgu = w256p.tile([128, DT, 256], F16, tag="w256", name="wgu")
                    nc.sync.dma_start(wgu[:], gu[it])
                    sc = psum.tile([128, 3, 512], F32, tag="sc", bufs=2,
                                   name="scm")
                    for dt in range(DT):
                        nc.tensor.matmul(sc[:, 0, :], wgu[:, dt, 0:128],
                                         hb[:, dt, :],
                                         start=(dt == 0), stop=(dt == DT - 1))
                    for dt in range(DT):
                        nc.tensor.matmul(sc[:, 1, :], wgu[:, dt, 128:256],
                                         hb[:, dt, :],
                                         start=(dt == 0), stop=(dt == DT - 1))
                    sg = sp.tile([128, 512], F16, tag="sg", bufs=1, name="sg")
                    nc.scalar.activation(sg[:], sc[:, 0, :], AF.Silu)
                    nc.vector.tensor_tensor(act[:, it, :], sc[:, 1, :], sg[:], MUL)
                    if it == IT - 1:
                        preload(AF.Sqrt)
                for dt2 in range(DT):
                    ps = psum.tile([128, 3, 512], F32, tag="sc", bufs=2, name="psd")[:, 0, :]
                    for half in range(2):
                        i0 = half * 11
                        w = wdp.tile([128, 11, 128], F16, tag="wd", name="wdn")
                        nc.sync.dma_start(w[:], dn[dt2, :, i0 : i0 + 11])
                        for it in range(11):
                            nc.tensor.matmul(ps[:], w[:, it, :],
                                             act[:, i0 + it, :],
                                             start=(i0 + it == 0),
                                             stop=(i0 + it == IT - 1))
                    nc.vector.tensor_tensor(hs[:, dt2, :], hb[:, dt2, :],
                                            ps[:], ADD)
                _rmsnorm(end_out)
                preload(AF.Exp)

            n_calls = len(level_calls)
            for ci, lvl in enumerate(level_calls):
                if lvl == "L":
                    state = zL
                    for dt in range(DT):
                        eng = nc.vector if dt % 2 == 0 else nc.gpsimd
                        eng.tensor_tensor(hb[:, dt, :], zL[:, dt, :],
                                          zH[:, dt, :], ADD)
                        eng2 = nc.gpsimd if dt % 2 == 0 else nc.vector
                        eng2.tensor_tensor(hb[:, dt, :], hb[:, dt, :],
                                           emb[:, dt, :], ADD)
                    pre = "L"
                else:
                    state = zH
                    for dt in range(DT):
                        eng = nc.vector if dt % 2 == 0 else nc.gpsimd
                        eng.tensor_tensor(hb[:, dt, :], zH[:, dt, :],
                                          zL[:, dt, :], ADD)
                    pre = "H"
                for i in range(2):
                    block(
                        inp[f"{pre}_wqT"][i], inp[f"{pre}_wkT"][i],
                        inp[f"{pre}_wvT"][i], inp[f"{pre}_woT"][i],
                        inp[f"{pre}_guT"][i], inp[f"{pre}_dnT"][i],
                        mid_out=hb, end_out=(hb if i == 0 else state),
                    )

            nc.sync.dma_start(
                out_t.rearrange("(dt p) t -> p dt t", p=128), zH[:])

    nc.compile()
    return nc


def _prep_weights(inputs):
    bf = np.float16
    w = {}
    for pre in ("L", "H"):
        # [out, in] torch-style weights -> pre-tiled [L, ot, p(in), dt(in), m]
        for nm, src in [("wqT", "wq"), ("wkT", "wk"), ("woT", "wo")]:
            a = np.asarray(inputs[f"{pre}_{src}"])  # [2, D, D] = [l, o, i]
            t = a.reshape(2, DT, 128, DT, 128)       # [l, ot, m, dt, p]
            w[f"{pre}_{nm}"] = np.ascontiguousarray(
                t.transpose(0, 1, 4, 3, 2)).astype(bf)
        a = np.asarray(inputs[f"{pre}_wv"])          # [2, D, D]
        t = a.reshape(2, 4, 256, DT, 128)            # [l, oc, m, dt, p]
        w[f"{pre}_wvT"] = np.ascontiguousarray(
            t.transpose(0, 1, 4, 3, 2)).astype(bf)
        g = np.asarray(inputs[f"{pre}_gu"])          # [2, 2*INTER, D]
        gate = g[:, :INTER].reshape(2, IT, 128, DT, 128)
        up = g[:, INTER:].reshape(2, IT, 128, DT, 128)
        gu = np.concatenate([gate, up], axis=2)      # [l, it, 256(m), dt, p]
        w[f"{pre}_guT"] = np.ascontiguousarray(
            gu.transpose(0, 1, 4, 3, 2)).astype(bf)  # [l, it, p, dt, 256]
        d = np.asarray(inputs[f"{pre}_dn"])          # [2, D, INTER]
        t = d.reshape(2, DT, 128, IT, 128)           # [l, ot, m, it, p]
        w[f"{pre}_dnT"] = np.ascontiguousarray(
            t.transpose(0, 1, 4, 3, 2)).astype(bf)   # [l, ot, p, it, 128]
    cos = np.asarray(inputs["cos"])  # [S, 64]
    sin = np.asarray(inputs["sin"])
    cosT = np.tile(cos.T, (2, 1)).astype(np.float16)          # [128, S]
    sinT_s = sin.T.copy()
    sinT_s[:32] *= -1.0
    sinT = np.tile(sinT_s, (2, 1)).astype(np.float16)          # [128, S]
    # row-swizzled sin: sinx[p] = sinT[sigma(p)], sigma swaps 32-row halves
    # within each 64-row head block, so the shifted rope multiplies read
    # in0/in1 at the same base partition (SBUF-SBUF ops require it).
    sinx = sinT.copy()
    for b in (0, 64):
        sinx[b:b+32], sinx[b+32:b+64] = sinT[b+32:b+64].copy(), sinT[b:b+32].copy()
    return w, cosT, sinT, sinx


def kernel(**inputs):
    key = "nc"
    if key not in _CACHE:
        _CACHE[key] = build_kernel()
    nc = _CACHE[key]

    w, cosT, sinT, sinx = _prep_weights(inputs)
    zL = np.asarray(inputs["z_L"], np.float32).astype(np.float16)
    zH = np.asarray(inputs["z_H"], np.float32).astype(np.float16)
    emb = np.asarray(inputs["input_emb"], np.float32).astype(np.float16)

    in_maps = []
    for c in range(8):
        b, half = c // 2, c % 2
        sl = slice(half * T, (half + 1) * T)
        m = {
            "zL": np.ascontiguousarray(zL[b].T[:, sl]),
            "zH": np.ascontiguousarray(zH[b].T[:, sl]),
            "emb": np.ascontiguousarray(emb[b].T[:, sl]),
            "cosT": np.ascontiguousarray(cosT[:, sl]),
            "sinxT": np.ascontiguousarray(sinx[:, sl]),
        }
        m.update(w)
        in_maps.append(m)

    trace = os.environ.get("HRM_TRACE", "0") == "1"
    res = run_bass_kernel_spmd(nc, in_maps, core_ids=list(range(8)), trace=trace)
    _CACHE["last_result"] = res

    out = np.empty((B, S, D), np.float32)
    for c in range(8):
        b, half = c // 2, c % 2
        out[b, half * T : (half + 1) * T, :] = (
            res.results[c]["zH_out"].astype(np.float32).T)
    return out


if __name__ == "__main__":
    rng = np.random.default_rng(0)
    ins = {
        "z_H": rng.standard_normal((B, S, D), np.float32),
        "z_L": rng.standard_normal((B, S, D), np.float32),
        "input_emb": rng.standard_normal((B, S, D), np.float32),
    }
    sd = 1.0 / np.sqrt(D)
    si = 1.0 / np.sqrt(INTER)
    for pre in ("L", "H"):
        for nm, shape, s in [("wq", (2, D, D), sd), ("wk", (2, D, D), sd),
                             ("wv", (2, D, D), sd), ("wo", (2, D, D), sd),
                             ("gu", (2, 2 * INTER, D), sd), ("dn", (2, D, INTER), si)]:
            ins[f"{pre}_{nm}"] = rng.standard_normal(shape, np.float32) * s
    inv = 1.0 / (10000.0 ** (np.arange(0, HD, 2, np.float32) / HD))
    fr = np.outer(np.arange(S, np.float32), inv)
    e = np.concatenate([fr, fr], -1)
    ins["cos"], ins["sin"] = np.cos(e).astype(np.float32), np.sin(e).astype(np.float32)
    out = kernel(**ins)
    print("out", out.shape, out.dtype, np.abs(out).mean())


# revision 20
# speedup vs baseline: 1.2090x; 1.2090x over previous
"""HRM dense-transformer kernel for 8 trn2 NeuronCores.

Sharding: data-parallel over batch (4) x sequence-parallel (2).
Core c handles batch b=c//2, token half h=c%2 (512 tokens).
Per block each core computes q/k/v for its own tokens, all-gathers
k^T and v (within its pair), then computes attention for its 512
queries over all 1024 keys. Activations are feature-major
([feature(part), token(free)]); scores are computed transposed [tk, tq]
so the softmax sum is a ones-matmul partition reduction. V is computed
token-major by swapping matmul operands, with a ones column appended
(M=65) so the softmax denominator falls out of the PV matmul.

Perf structure (v3):
- residual stream is fp16 (zL/zH/emb/hb); fp32 only in the pre-norm
  scratch hs. Norm mults write the f16 state directly (no casts), split
  across DVE and Pool.
- rope: ACT copies the projection psum to fp16 SBUF, then all-fp16
  elementwise ops run at 2x split across DVE+Pool.
- scores land in [128,2,512] 2-bank PSUM chunks, exp'd in [128,1024]
  batches. Softmax normalize is deferred per head-pair: denominators at
  partitions 0/32, fast-recip, one bf16 K=64 selector matmul broadcast
  (bf16 for range: 1/den underflows fp16).
- rmsnorm: square on Pool (keeps ACT tables stable), Sqrt on ACT +
  fast-recip, fp16 K=1 broadcast matmul.
- ACT table switches (Exp/Sqrt/Silu) are prefetched with dummy [1,1]
  activations during PE-heavy phases.
- weights pre-tiled host-side: every weight DMA reads 2KB contiguous
  per partition; gate+up column blocks interleave into one DMA.
"""

import os
import sys

sys.path.insert(0, "/opt/trn_rl_repo")

import ml_dtypes
import numpy as np

import concourse.bass as bass
import concourse.mybir as mybir
import concourse.tile as tile
from concourse import bacc
from concourse.bass_utils import run_bass_kernel_spmd

F32 = mybir.dt.float32
F16 = mybir.dt.float16
BF16 = mybir.dt.bfloat16
AF = mybir.ActivationFunctionType
MUL = mybir.AluOpType.mult
ADD = mybir.AluOpType.add

B, S, D, NH, HD = 4, 1024, 1024, 16, 64
INTER = 2816
T = S // 2              # own tokens per core
DT = D // 128           # 8 d-tiles
IT = INTER // 128       # 22 inter tiles
VF = NH * (HD + 1)      # 1040, v_aug feature width
EPS = 1e-5
KN = T * S              # kT elems (own): 1024 x 512
VN = T * VF             # v_aug elems (own): 512 x 1040
RG = [[0, 1], [2, 3], [4, 5], [6, 7]]

N_LEVEL_CALLS = int(os.environ.get("HRM_LEVEL_CALLS", "6"))

_CACHE = {}


def build_kernel():
    nc = bacc.Bacc("TRN2", target_bir_lowering=False, debug=False, num_devices=8)

    inp = {}
    for nm, shape, dt in [
        ("zL", [D, T], F16), ("zH", [D, T], F16), ("emb", [D, T], F16),
        ("cosT", [128, T], F16), ("sinxT", [128, T], F16),
        # pre-tiled weights: [layer, out-tile, 128(p=in), in-tile, m]
        ("L_wqT", [2, DT, 128, DT, 128], F16),
        ("L_wkT", [2, DT, 128, DT, 128], F16),
        ("L_wvT", [2, 4, 128, DT, 256], F16),
        ("L_woT", [2, DT, 128, DT, 128], F16),
        ("L_guT", [2, IT, 128, DT, 256], F16),
        ("L_dnT", [2, DT, 128, IT, 128], F16),
        ("H_wqT", [2, DT, 128, DT, 128], F16),
        ("H_wkT", [2, DT, 128, DT, 128], F16),
        ("H_wvT", [2, 4, 128, DT, 256], F16),
        ("H_woT", [2, DT, 128, DT, 128], F16),
        ("H_guT", [2, IT, 128, DT, 256], F16),
        ("H_dnT", [2, DT, 128, IT, 128], F16),
    ]:
        inp[nm] = nc.dram_tensor(nm, shape, dt, kind="ExternalInput")
    out_t = nc.dram_tensor("zH_out", [D, T], F16, kind="ExternalOutput")

    seq = os.environ.get("HRM_SEQ", "")
    if seq:
        level_calls = list(seq)
    else:
        level_calls = (["L", "L", "H"] * 2)[:N_LEVEL_CALLS]

    with tile.TileContext(nc) as tc:
        with (
            tc.tile_pool(name="state", bufs=1) as st,
            tc.tile_pool(name="sp", bufs=2) as sp,
            tc.tile_pool(name="big", bufs=2) as bigp,
            tc.tile_pool(name="w128", bufs=3) as w128p,
            tc.tile_pool(name="w256", bufs=2) as w256p,
            tc.tile_pool(name="wd", bufs=2) as wdp,
            tc.tile_pool(name="pt", bufs=2) as ptp,
            tc.tile_pool(name="psum", bufs=1, space="PSUM") as psum,
            tc.tile_pool(name="dram", bufs=2, space="DRAM") as dram,
        ):
            zL = st.tile([128, DT, T], F16, name="zL_sb")
            zH = st.tile([128, DT, T], F16, name="zH_sb")
            emb = st.tile([128, DT, T], F16, name="emb_sb")
            hs = st.tile([128, DT, T], F32, name="hs")
            cos = st.tile([128, T], F16, name="cos_sb")
            sinx = st.tile([128, T], F16, name="sinx_sb")
            hb = st.tile([128, DT, T], F16, name="hb")
            qT = st.tile([128, DT, T], F16, name="qT")
            kst = st.tile([128, DT, T], F16, name="kst")
            vst = st.tile([128, 4, VF], BF16, name="vst")
            oT = st.tile([128, DT, T], F16, name="oT")
            ones128 = st.tile([128, 1], F16, name="ones128")
            ones1f = st.tile([1, 128], F16, name="ones1f")
            sel64 = st.tile([64, 128], BF16, name="sel64")
            den2 = st.tile([64, 512], F32, name="den2")
            denr = st.tile([64, 512], F32, name="denr")
            denr16 = st.tile([64, 512], BF16, name="denr16")
            epsc = st.tile([1, 1], F32, name="epsc")
            dum = st.tile([1, 1], F32, name="dum")

            nc.sync.dma_start(zL[:], inp["zL"].rearrange("(dt p) t -> p dt t", p=128))
            nc.sync.dma_start(zH[:], inp["zH"].rearrange("(dt p) t -> p dt t", p=128))
            nc.sync.dma_start(emb[:], inp["emb"].rearrange("(dt p) t -> p dt t", p=128))
            nc.sync.dma_start(cos[:], inp["cosT"][:])
            nc.sync.dma_start(sinx[:], inp["sinxT"][:])
            nc.vector.memset(ones128[:], 1.0)
            nc.vector.memset(ones1f[:], 1.0)
            nc.vector.memset(epsc[:], EPS)
            nc.vector.memset(dum[:], 1.0)
            # selector for the softmax-denominator broadcast: row 0 feeds
            # out partitions 0-63 (even head), row 32 feeds 64-127 (odd).
            # den rows sit at partitions 0/32 (engine writes need 32-aligned
            # partition bases); unused rows stay at the 1.0 init so the
            # zero-weighted matmul columns never see NaN/Inf.
            nc.vector.memset(sel64[:], 0.0)
            nc.vector.memset(sel64[0:1, 0:64], 1.0)
            nc.vector.memset(sel64[32:33, 64:128], 1.0)
            nc.vector.memset(den2[:], 1.0)
            nc.vector.memset(denr[:], 1.0)
            nc.vector.memset(denr16[:], 1.0)
            # ones columns of v_aug (written once; data copies avoid them)
            nc.vector.memset(
                vst.rearrange("p tt (h c) -> p tt h c", c=HD + 1)[:, :, :, HD : HD + 1],
                1.0,
            )

            def preload(func):
                # dummy activation to pull the act table in early
                nc.scalar.activation(dum[:], dum[:], func)

            def _rope(ps, out_ap, pool_assist=False):
                """out = ps*cos + rotate_half(ps)*sin.

                partition dim = 2 heads x 64; sinT rows pre-negated for the
                first half of each head. ACT stages the psum to fp16 so the
                elementwise ops run all-fp16 at 2x, split DVE/Pool.
                """
                qr = sp.tile([128, 512], F16, tag="qr", name="qr")
                nc.scalar.copy(qr[:], ps[:])
                t1 = sp.tile([128, 512], F16, tag="rope1", name="rope1")
                t2 = sp.tile([128, 512], F16, tag="rope2", name="rope2")
                nc.vector.tensor_tensor(t1[:], qr[:], cos[:], MUL)
                nc.vector.tensor_tensor(t2[0:32, :], qr[32:64, :],
                                        sinx[32:64, :], MUL)
                nc.vector.tensor_tensor(t2[32:64, :], qr[0:32, :],
                                        sinx[0:32, :], MUL)
                e2 = nc.gpsimd if pool_assist else nc.vector
                e2.tensor_tensor(t2[64:96, :], qr[96:128, :],
                                 sinx[96:128, :], MUL)
                e2.tensor_tensor(t2[96:128, :], qr[64:96, :],
                                 sinx[64:96, :], MUL)
                if pool_assist:
                    nc.gpsimd.tensor_tensor(out_ap, t1[:], t2[:], ADD)
                else:
                    nc.vector.tensor_add(out=out_ap, in0=t1[:], in1=t2[:])

            def _rmsnorm(out_tile):
                """out = hs * rsqrt(mean(hs^2)+eps) in fp16.

                square on Pool (keeps ACT table stable), partition-reduce via
                ones matmul, Sqrt on ACT + fast reciprocal, fp16 K=1
                broadcast matmul, mults split DVE/Pool."""
                sst = psum.tile([65, 512], F32, tag="pv", bufs=2, name="sst")
                ss = sst[0:1, :]
                for dt in range(DT):
                    r2 = sp.tile([128, 512], F16, tag="r2", name="r2")
                    nc.gpsimd.tensor_tensor(r2[:], hs[:, dt, :], hs[:, dt, :],
                                            MUL)
                    nc.tensor.matmul(ss, ones128[:], r2[:], start=(dt == 0),
                                     stop=(dt == DT - 1))
                sq = sp.tile([1, 512], F32, tag="sq", bufs=1, name="sq")
                nc.scalar.activation(sq[:], ss, AF.Sqrt, bias=epsc[:],
                                     scale=1.0 / D)
                rstdf = sp.tile([1, 512], F32, tag="rstdf", bufs=1, name="rstdf")
                nc.vector.reciprocal_approx_fast(out=rstdf[:], in_=sq[:])
                rstd = sp.tile([1, 512], F16, tag="rstd", bufs=1, name="rstd")
                nc.vector.tensor_copy(out=rstd[:], in_=rstdf[:])
                pb = psum.tile([128, 512], F32, tag="mm", bufs=2, name="pbn")
                nc.tensor.matmul(pb[:], ones1f[:], rstd[:], start=True, stop=True)
                bc = sp.tile([128, 512], F16, tag="bcn", bufs=1, name="bcn")
                nc.vector.tensor_copy(out=bc[:], in_=pb[:])
                for dt in range(DT):
                    eng = nc.vector if dt % 2 == 0 else nc.gpsimd
                    eng.tensor_tensor(out_tile[:, dt, :], hs[:, dt, :], bc[:],
                                      MUL)

            def block(wq, wk, wv, wo, gu, dn, mid_out, end_out):
                """One HRMBlock on hb; attn rmsnorm -> mid_out(=hb),
                mlp rmsnorm -> end_out (hb or the level state tile)."""
                gin_k = dram.tile([KN], F16, tag="gin_k", name="gin_k")
                gout_k = dram.tile([2 * KN], F16, tag="gout_k", name="gout_k")
                gin_v = dram.tile([VN], BF16, tag="gin_v", name="gin_v")
                gout_v = dram.tile([2 * VN], BF16, tag="gout_v", name="gout_v")

                # ---- k projection + rope ----
                for ot in range(DT):
                    w = w128p.tile([128, DT, 128], F16, tag="w128", name="wk")
                    nc.sync.dma_start(w[:], wk[ot])
                    ps = psum.tile([128, 512], F32, tag="mm", bufs=2, name="psk")
                    for dt in range(DT):
                        nc.tensor.matmul(ps[:], w[:, dt, :], hb[:, dt, :],
                                         start=(dt == 0), stop=(dt == DT - 1))
                    _rope(ps, kst[:, ot, :])
                # ---- send + gather k (overlaps v/q projection) ----
                nc.sync.dma_start(
                    gin_k[:].rearrange("(dt p t) -> p dt t", p=128, t=T), kst[:])
                nc.gpsimd.collective_compute(
                    "AllGather", mybir.AluOpType.bypass, replica_groups=RG,
                    ins=[gin_k.opt()], outs=[gout_k.opt()])
                # ---- v projection (token-major) ----
                vsr = vst.rearrange("p tt (hh c) -> p tt hh c", c=HD + 1)
                for oc in range(4):
                    w = w256p.tile([128, DT, 256], F16, tag="w256", name="wv")
                    nc.sync.dma_start(w[:], wv[oc])
                    for tt in range(4):
                        ps = psum.tile([128, 512], F32, tag="mm", bufs=2,
                                       name="psv")[:, 0:256]
                        for dt in range(DT):
                            nc.tensor.matmul(
                                ps[:], hb[:, dt, tt * 128 : (tt + 1) * 128],
                                w[:, dt, :], start=(dt == 0), stop=(dt == DT - 1))
                        nc.vector.tensor_copy(
                            out=vsr[:, tt, oc * 4 : (oc + 1) * 4, 0:HD],
                            in_=ps.rearrange("p (hh c) -> p hh c", c=HD))
                # ---- send + gather v (k already in flight) ----
                nc.sync.dma_start(
                    gin_v[:].rearrange("(tt p f) -> p tt f", p=128, f=VF), vst[:])
                nc.gpsimd.collective_compute(
                    "AllGather", mybir.AluOpType.bypass, replica_groups=RG,
                    ins=[gin_v.opt()], outs=[gout_v.opt()])
                # ---- q projection + rope (overlaps gather) ----
                for ot in range(DT):
                    w = w128p.tile([128, DT, 128], F16, tag="w128", name="wq")
                    nc.sync.dma_start(w[:], wq[ot])
                    ps = psum.tile([128, 512], F32, tag="mm", bufs=2, name="psq")
                    for dt in range(DT):
                        nc.tensor.matmul(ps[:], w[:, dt, :], hb[:, dt, :],
                                         start=(dt == 0), stop=(dt == DT - 1))
                    _rope(ps, qT[:, ot, :])
                # ---- load gathered k/v ----
                kTf = bigp.tile([128, DT, S], F16, tag="big", name="kTf")
                vf = bigp.tile([128, DT, VF], BF16, tag="big", name="vf")
                for r in range(2):
                    nc.sync.dma_start(
                        kTf[:, :, r * T : (r + 1) * T],
                        gout_k[r * KN : (r + 1) * KN].rearrange(
                            "(dt p t) -> p dt t", p=128, t=T))
                    nc.sync.dma_start(
                        vf[:, 4 * r : 4 * r + 4, :],
                        gout_v[r * VN : (r + 1) * VN].rearrange(
                            "(tt p f) -> p tt f", p=128, f=VF))
                # ---- attention ----
                for ot in range(DT):
                    # combined [sub, kt] prob tile; each psum chunk holds the
                    # two subs' scores at one kt so the paired K=64 matmuls
                    # run concurrently on PE row-quadrants (0,0)+(64,0).
                    pt = ptp.tile([128, 2, DT, 512], BF16, tag="pt", bufs=2,
                                  name="pt")
                    for kt in range(DT):
                        sc = psum.tile([128, 2, 512], F32, tag="sc", bufs=2,
                                       name="sc")
                        for sub in range(2):
                            bp = sub * 64
                            nc.tensor.matmul(
                                sc[:, sub, :],
                                kTf[bp : bp + 64, ot, kt * 128 : (kt + 1) * 128],
                                qT[bp : bp + 64, ot, :],
                                start=True, stop=True, tile_position=(bp, 0))
                        nc.scalar.activation(
                            pt[:, :, kt, :], sc[:], AF.Exp, scale=0.125)
                    pvs = []
                    for sub in range(2):
                        hh = ot * 2 + sub
                        pv = psum.tile([65, 512], F32, tag="pv", bufs=2, name="pv")
                        for kt in range(DT):
                            nc.tensor.matmul(
                                pv[:],
                                vf[:, kt, hh * (HD + 1) : (hh + 1) * (HD + 1)],
                                pt[:, sub, kt, :],
                                start=(kt == 0), stop=(kt == DT - 1))
                        nc.vector.tensor_copy(out=den2[sub * 32 : sub * 32 + 1, :],
                                              in_=pv[64:65, :])
                        pvs.append(pv)
                    if ot == DT - 1:
                        preload(AF.Sqrt)
                    nc.vector.reciprocal_approx_fast(out=denr[:], in_=den2[:])
                    nc.vector.tensor_copy(out=denr16[:], in_=denr[:])
                    pb = psum.tile([128, 512], F32, tag="mm", bufs=2, name="pbc")
                    nc.tensor.matmul(pb[:], sel64[:], denr16[:], start=True,
                                     stop=True)
                    bc = sp.tile([128, 512], BF16, tag="bc", bufs=1, name="bc")
                    nc.vector.tensor_copy(out=bc[:], in_=pb[:])
                    for sub in range(2):
                        bp = sub * 64
                        nc.vector.tensor_tensor(
                            oT[bp : bp + 64, ot, :], pvs[sub][0:HD, :],
                            bc[bp : bp + 64, :], MUL)
                # ---- o projection + residual ----
                for dt2 in range(DT):
                    w = w128p.tile([128, DT, 128], F16, tag="w128", name="wo")
                    nc.sync.dma_start(w[:], wo[dt2])
                    ps = psum.tile([128, 512], F32, tag="mm", bufs=2, name="pso")
                    for et in range(DT):
                        nc.tensor.matmul(ps[:], w[:, et, :], oT[:, et, :],
                                         start=(et == 0), stop=(et == DT - 1))
                    nc.vector.tensor_tensor(hs[:, dt2, :], hb[:, dt2, :],
                                            ps[:], ADD)
                _rmsnorm(mid_out)
                preload(AF.Silu)
                # ---- MLP ----
                act = bigp.tile([128, IT, 512], F16, tag="big", name="act")
                for it in range(IT):
                    wgu = w256p.tile([128, DT, 256], F16, tag="w256", name="wgu")
                    nc.sync.dma_start(wgu[:], gu[it])
                    sc = psum.tile([128, 2, 512], F32, tag="sc", bufs=2,
                                   name="scm")
                    for dt in range(DT):
                        nc.tensor.matmul(sc[:, 0, :], wgu[:, dt, 0:128],
                                         hb[:, dt, :],
                                         start=(dt == 0), stop=(dt == DT - 1))
                    for dt in range(DT):
                        nc.tensor.matmul(sc[:, 1, :], wgu[:, dt, 128:256],
                                         hb[:, dt, :],
                                         start=(dt == 0), stop=(dt == DT - 1))
                    sg = sp.tile([128, 512], F16, tag="sg", bufs=1, name="sg")
                    nc.scalar.activation(sg[:], sc[:, 0, :], AF.Silu)
                    nc.vector.tensor_tensor(act[:, it, :], sc[:, 1, :], sg[:], MUL)
                    if it == IT - 1:
                        preload(AF.Sqrt)
                for dt2 in range(DT):
                    ps = psum.tile([128, 512], F32, tag="mm", bufs=2, name="psd")
                    for half in range(2):
                        i0 = half * 11
                        w = wdp.tile([128, 11, 128], F16, tag="wd", name="wdn")
                        nc.sync.dma_start(w[:], dn[dt2, :, i0 : i0 + 11])
                        for it in range(11):
                            nc.tensor.matmul(ps[:], w[:, it, :],
                                             act[:, i0 + it, :],
                                             start=(i0 + it == 0),
                                             stop=(i0 + it == IT - 1))
                    nc.vector.tensor_tensor(hs[:, dt2, :], hb[:, dt2, :],
                                            ps[:], ADD)
                _rmsnorm(end_out)
                preload(AF.Exp)

            n_calls = len(level_calls)
            for ci, lvl in enumerate(level_calls):
                if lvl == "L":
                    state = zL
                    for dt in range(DT):
                        eng = nc.vector if dt % 2 == 0 else nc.gpsimd
                        eng.tensor_tensor(hb[:, dt, :], zL[:, dt, :],
                                          zH[:, dt, :], ADD)
                        eng2 = nc.gpsimd if dt % 2 == 0 else nc.vector
                        eng2.tensor_tensor(hb[:, dt, :], hb[:, dt, :],
                                           emb[:, dt, :], ADD)
                    pre = "L"
                else:
                    state = zH
                    for dt in range(DT):
                        eng = nc.vector if dt % 2 == 0 else nc.gpsimd
                        eng.tensor_tensor(hb[:, dt, :], zH[:, dt, :],
                                          zL[:, dt, :], ADD)
                    pre = "H"
                for i in range(2):
                    block(
                        inp[f"{pre}_wqT"][i], inp[f"{pre}_wkT"][i],
                        inp[f"{pre}_wvT"][i], inp[f"{pre}_woT"][i],
                        inp[f"{pre}_guT"][i], inp[f"{pre}_dnT"][i],
                        mid_out=hb, end_out=(hb if i == 0 else state),
                    )

            nc.sync.dma_start(
                out_t.rearrange("(dt p) t -> p dt t", p=128), zH[:])

    nc.compile()
    return nc


def _prep_weights(inputs):
    bf = np.float16
    w = {}
    for pre in ("L", "H"):
        # [out, in] torch-style weights -> pre-tiled [L, ot, p(in), dt(in), m]
        for nm, src in [("wqT", "wq"), ("wkT", "wk"), ("woT", "wo")]:
            a = np.asarray(inputs[f"{pre}_{src}"])  # [2, D, D] = [l, o, i]
            t = a.reshape(2, DT, 128, DT, 128)       # [l, ot, m, dt, p]
            w[f"{pre}_{nm}"] = np.ascontiguousarray(
                t.transpose(0, 1, 4, 3, 2)).astype(bf)
        a = np.asarray(inputs[f"{pre}_wv"])          # [2, D, D]
        t = a.reshape(2, 4, 256, DT, 128)            # [l, oc, m, dt, p]
        w[f"{pre}_wvT"] = np.ascontiguousarray(
            t.transpose(0, 1, 4, 3, 2)).astype(bf)
        g = np.asarray(inputs[f"{pre}_gu"])          # [2, 2*INTER, D]
        gate = g[:, :INTER].reshape(2, IT, 128, DT, 128)
        up = g[:, INTER:].reshape(2, IT, 128, DT, 128)
        gu = np.concatenate([gate, up], axis=2)      # [l, it, 256(m), dt, p]
        w[f"{pre}_guT"] = np.ascontiguousarray(
            gu.transpose(0, 1, 4, 3, 2)).astype(bf)  # [l, it, p, dt, 256]
        d = np.asarray(inputs[f"{pre}_dn"])          # [2, D, INTER]
        t = d.reshape(2, DT, 128, IT, 128)           # [l, ot, m, it, p]
        w[f"{pre}_dnT"] = np.ascontiguousarray(
            t.transpose(0, 1, 4, 3, 2)).astype(bf)   # [l, ot, p, it, 128]
    cos = np.asarray(inputs["cos"])  # [S, 64]
    sin = np.asarray(inputs["sin"])
    cosT = np.tile(cos.T, (2, 1)).astype(np.float16)          # [128, S]
    sinT_s = sin.T.copy()
    sinT_s[:32] *= -1.0
    sinT = np.tile(sinT_s, (2, 1)).astype(np.float16)          # [128, S]
    # row-swizzled sin: sinx[p] = sinT[sigma(p)], sigma swaps 32-row halves
    # within each 64-row head block, so the shifted rope multiplies read
    # in0/in1 at the same base partition (SBUF-SBUF ops require it).
    sinx = sinT.copy()
    for b in (0, 64):
        sinx[b:b+32], sinx[b+32:b+64] = sinT[b+32:b+64].copy(), sinT[b:b+32].copy()
    return w, cosT, sinT, sinx


def kernel(**inputs):
    key = "nc"
    if key not in _CACHE:
        _CACHE[key] = build_kernel()
    nc = _CACHE[key]

    w, cosT, sinT, sinx = _prep_weights(inputs)
    zL = np.asarray(inputs["z_L"], np.float32).astype(np.float16)
    zH = np.asarray(inputs["z_H"], np.float32).astype(np.float16)
    emb = np.asarray(inputs["input_emb"], np.float32).astype(np.float16)

    in_maps = []
    for c in range(8):
        b, half = c // 2, c % 2
        sl = slice(half * T, (half + 1) * T)
        m = {
            "zL": np.ascontiguousarray(zL[b].T[:, sl]),
            "zH": np.ascontiguousarray(zH[b].T[:, sl]),
            "emb": np.ascontiguousarray(emb[b].T[:, sl]),
            "cosT": np.ascontiguousarray(cosT[:, sl]),
            "sinxT": np.ascontiguousarray(sinx[:, sl]),
        }
        m.update(w)
        in_maps.append(m)

    trace = os.environ.get("HRM_TRACE", "0") == "1"
    res = run_bass_kernel_spmd(nc, in_maps, core_ids=list(range(8)), trace=trace)
    _CACHE["last_result"] = res

    out = np.empty((B, S, D), np.float32)
    for c in range(8):
        b, half = c // 2, c % 2
        out[b, half * T : (half + 1) * T, :] = (
            res.results[c]["zH_out"].astype(np.float32).T)
    return out


if __name__ == "__main__":
    rng = np.random.default_rng(0)
    ins = {
        "z_H": rng.standard_normal((B, S, D), np.float32),
        "z_L": rng.standard_normal((B, S, D), np.float32),
        "input_emb": rng.standard_normal((B, S, D), np.float32),
    }
    sd = 1.0 / np.sqrt(D)
    si = 1.0 / np.sqrt(INTER)
    for pre in ("L", "H"):
        for nm, shape, s in [("wq", (2, D, D), sd), ("wk", (2, D, D), sd),
                             ("wv", (2, D, D), sd), ("wo", (2, D, D), sd),
                             ("gu", (2, 2 * INTER, D), sd), ("dn", (2, D, INTER), si)]:
            ins[f"{pre}_{nm}"] = rng.standard_normal(shape, np.float32) * s
    inv = 1.0 / (10000.0 ** (np.arange(0, HD, 2, np.float32) / HD))
    fr = np.outer(np.arange(S, np.float32), inv)
    e = np.concatenate([fr, fr], -1)
    ins["cos"], ins["sin"] = np.cos(e).astype(np.float32), np.sin(e).astype(np.float32)
    out = kernel(**ins)
    print("out", out.shape, out.dtype, np.abs(out).mean())
